# revision 31
# baseline (speedup 1.0000x reference)
"""Trainium2 Bass kernel for nn_AutoSelectAttention (parametric Gaussian span scores).

Computes y[b,m,k] = -(((x[k] + mean[b,m]) / (softness[b,m] + EPS))**2) + intercept[b,m]
for x[k] = k - (L-1), k in [0, 2L-1).

Sharding: the fused batch*heads dim (32) is split 4-per-core across 8 NeuronCores;
each core's [4*1024, 2047] output band is independent (no collectives).

Per-core schedule (DMA-write-roofline bound, ~33.5 MB f32 out per core):
  - host precomputes per-token planes [mean, -1/(s+eps)^2, intercept] -> one
    small input DMA; its completion (~9.6us incl. fixed preamble) gates compute.
  - x grid fp16 (exact for |int| <= 2048) built as one 512-col gpsimd iota +
    three DVE +const shifts so it's ready before the planes semaphore.
  - per block: ACT Square (z2 = (x+mean)^2, f32) then one DVE tensor_scalar
    (y = z2*ninv2 + intercept) into a grouped output tile.
  - output DRAM is group-contiguous (y1[2,128,W] singles, y2[15,128,2W]
    pairs): each group is one fully contiguous 1-2MB DRAM region whose
    per-partition descriptor is 8188/16376 contiguous bytes (16KB
    descriptors run at ~426 GB/s, within 1% of the write-side wall).  Two
    singles start the write stream at ~14us; all-pairs steady state keeps
    the stream ACT- or DMA-paced with no readiness stalls at any compute
    cadence, and the fine completion granularity softens slow-engine runs.
  - all DMAs keep the full 128 partitions, split by the descriptor
    generator over 16 SDMA engines x 8 partitions.  SDMA engine 15 is
    stochastically 20-50% slower (shared-port contention, costs up to
    ~15us on sick runs), but every alternative that sidesteps it --
    112/120/124-partition DMAs, mixed shapes, per-partition splits --
    measured equal or worse (15 engines / chip-wide slowdowns), so full
    128-partition uniform shape is the best expected-value config.
"""

import sys

import numpy as np

for _p in ("/opt/trn_rl_repo", "/root/.axon_site", "/opt/pypackages"):
    if _p not in sys.path:
        sys.path.append(_p)

L = 1024
W = 2 * L - 1  # 2047
BH = 32
M = 1024
EPS = 1e-5
NCORES = 8
BH_SH = BH // NCORES  # 4
ROWS = BH_SH * M  # 4096 tokens per core
P = 128
NBLK = ROWS // P  # 32 blocks of 128 tokens
HW_ROWS = NBLK * P  # 4096; no host remainder
GROUPS = [1, 1] + [2] * 15
assert sum(GROUPS) == NBLK

_NC_CACHE = {}


def _build_nc():
    import concourse.bacc as bacc
    import concourse.tile as tile
    from concourse import mybir

    f32 = mybir.dt.float32
    f16 = mybir.dt.float16
    Sq = mybir.ActivationFunctionType.Square

    nc = bacc.Bacc("TRN2", target_bir_lowering=False, debug=False)
    # planes[p, 0, k] = mean, [p, 1, k] = -1/(softness+EPS)^2, [p, 2, k] =
    # intercept for token t = k*120 + p (host-precomputed).
    planes = nc.dram_tensor("planes", [P, 3, NBLK], f32, kind="ExternalInput").ap()
    # One output tensor per group size; group i of size g occupies one fully
    # contiguous DRAM region laid out [partition, g*W] so every partition's
    # descriptor is g*8188 contiguous bytes and partitions are adjacent
    # (y*[i, p, j*W+w] = out[token (k0+j)*128+p, w]).
    n_by_g = {g: GROUPS.count(g) for g in set(GROUPS)}
    youts = {
        g: nc.dram_tensor(f"y{g}", [n, P, g * W], f32, kind="ExternalOutput").ap()
        for g, n in sorted(n_by_g.items())
    }

    with tile.TileContext(nc) as tc:
        with (
            tc.tile_pool(name="const", bufs=1) as cpool,
            tc.tile_pool(name="work", bufs=3) as wpool,
            tc.tile_pool(name="o1", bufs=2) as o1pool,
            tc.tile_pool(name="o2", bufs=5) as o2pool,
        ):
            # Warmup ACTIVATE with no data dependencies: pulls the ~1.5us
            # Square table load to kernel start instead of serializing it
            # behind the planes DMA.
            warm = cpool.tile([P, 1], f32)
            one = nc.const_aps.tensor(1.0, (P, 1))
            nc.scalar.activation(warm[:], one, Sq, bias=0.0, scale=1.0)

            # x grid in fp16 (integers |x| <= 2047 are exact in fp16).
            xb = cpool.tile([P, 2 * L], f16)
            nc.gpsimd.iota(
                xb[:, 0:512],
                [[1, 512]],
                base=-(L - 1),
                channel_multiplier=0,
                allow_small_or_imprecise_dtypes=True,
            )
            for j in (1, 2, 3):
                nc.vector.tensor_scalar(
                    xb[:, j * 512 : (j + 1) * 512],
                    xb[:, 0:512],
                    float(j * 512),
                    None,
                    mybir.AluOpType.add,
                )

            spn = cpool.tile([P, 3, NBLK], f32)
            nc.sync.dma_start(spn[:], planes[:, :, :])

            pools = {1: o1pool, 2: o2pool}
            gidx = {g: 0 for g in n_by_g}
            k = 0
            for g in GROUPS:
                ot = pools[g].tile([P, g * W], f32)
                for j in range(g):
                    kk = k + j
                    # z2 = (x + mean)^2 on ACT (per-partition bias = mean)
                    z2 = wpool.tile([P, W], f32)
                    nc.scalar.activation(
                        z2[:], xb[:, 0:W], Sq, bias=spn[:, 0, kk : kk + 1], scale=1.0
                    )
                    # y = z2 * ninv2 + intercept on DVE (per-partition scalars)
                    nc.vector.tensor_scalar(
                        ot[:, j * W : (j + 1) * W],
                        z2[:],
                        spn[:, 1, kk : kk + 1],
                        spn[:, 2, kk : kk + 1],
                        mybir.AluOpType.mult,
                        mybir.AluOpType.add,
                    )
                i = gidx[g]
                nc.sync.dma_start(youts[g][i : i + 1, :, :], ot[:])
                gidx[g] += 1
                k += g
    nc.compile()
    return nc


def _get_nc():
    if "nc" not in _NC_CACHE:
        _NC_CACHE["nc"] = _build_nc()
    return _NC_CACHE["nc"]


def _make_in_maps(span: np.ndarray) -> list[dict]:
    span = np.ascontiguousarray(span, dtype=np.float32)
    in_maps = []
    for c in range(NCORES):
        flat = span[c * BH_SH : (c + 1) * BH_SH].reshape(ROWS, 3)
        # [blk, p, comp] with token t = blk*128 + p
        shard = flat[:HW_ROWS].reshape(NBLK, P, 3)
        mean = shard[:, :, 0].T  # [p, blk]
        soft = shard[:, :, 1].T.astype(np.float64)
        cept = shard[:, :, 2].T
        ninv2 = (-1.0 / (soft + EPS) ** 2).astype(np.float32)
        planes = np.ascontiguousarray(
            np.stack([mean, ninv2, cept], axis=1), dtype=np.float32
        )  # [128, 3, NBLK]
        in_maps.append({"planes": planes})
    return in_maps


def kernel(span: np.ndarray, _trace: bool = False, _tmpdir: str | None = None):
    from concourse.bass_utils import run_bass_kernel_spmd

    span = np.ascontiguousarray(span, dtype=np.float32)
    nc = _get_nc()
    in_maps = _make_in_maps(span)
    res = run_bass_kernel_spmd(
        nc,
        in_maps,
        core_ids=list(range(NCORES)),
        trace=_trace,
        tmpdir=_tmpdir,
    )
    # Reassemble each core's [ROWS, W] band from the group-contiguous
    # tensors: group i of size g holds [P, g, W] with token t = (k0+j)*128+p.
    shards = []
    for c, r in enumerate(res.results):
        band = np.empty((ROWS, W), np.float32)
        gidx = {g: 0 for g in set(GROUPS)}
        k = 0
        for g in GROUPS:
            i = gidx[g]
            arr = np.asarray(r[f"y{g}"]).reshape(-1, P, g * W)[i]
            band[k * P : (k + g) * P, :] = (
                arr.reshape(P, g, W).transpose(1, 0, 2).reshape(g * P, W)
            )
            gidx[g] += 1
            k += g
        shards.append(band.reshape(BH_SH, M, W))
    out = np.concatenate(shards, axis=0).astype(np.float32)
    if _trace:
        kernel.last_results = res
    return out
